# revision 23
# baseline (speedup 1.0000x reference)
"""BiLinearInteractionLayer (bilinear_type='all') Trainium2 Bass kernel.

Contract: kernel(inputs=[2048,40,64] f32, w=[64,64] f32) -> [2048, 49920] f32,
matching

    xw  = einsum('bfd,de->bfe', inputs, w)
    p   = xw[:, I, :] * inputs[:, J, :]   # (I, J) = triu_indices(40, k=1)
    out = p.reshape(B, -1)

Data-parallel over 8 NeuronCores: batch 2048 -> 8 x 256, W replicated.

Roofline: the kernel is HBM-DMA bound -- 51 MB of output writes per core
against a ~420 GB/s per-core DMA-engine ceiling (~125 us of pure queue
time; measured engines run 97-98% busy in steady state). The fight is to
keep every other engine comfortably below that:

  * f32 broadcast multiplies on DVE run at ~1.25 ns/elem/partition
    (no fast mode for 4-byte dtypes) = ~140 us -- they would pace the
    kernel. GpSimd shares DVE's SBUF ports, so offloading muls there is
    a wash (measured: both engines throttle to the same combined rate).
    Instead x and xw are converted once to fp16 (cheap, on ACT) and the
    muls run fp16-in/f32-out: half the read traffic, ~1.05-1.16 ns/elem,
    ~116 us -- production ~440 GB/s, just above the drain rate, with the
    f32 product written straight into the DMA staging tile (no second
    conversion pass). fp16 inputs keep relative error ~6e-4, far inside
    the 2e-2 gate.
  * Consecutive lead-fields are packed into contiguous output GROUPS
    (small groups first, capped ~4.6K f32/partition): one staging tile
    and one wide DMA per group (lines up to 18 KB), 16 DMAs per tile.
  * x streams in reverse-field chunks on the sync ring ahead of the
    write stream (the queue eats the reads while the mul pipeline
    ramps); transposes/matmuls/fp16 copies are emitted two groups ahead
    of the muls; tile1's chunks drip from the ACT stream mid-tile0.
  * No gpsimd SWDGE DMAs, and 7 staging buffers: DMA engine E79
    intermittently runs ~20% slow, and every group-completion semaphore
    (+16, one per engine) otherwise gates DVE on the straggler.
"""

import numpy as np
from contextlib import ExitStack

import concourse.bass as bass  # noqa: F401  (registers engines)
import concourse.bacc as bacc
import concourse.tile as tile
import concourse.mybir as mybir
from concourse.bass_utils import run_bass_kernel_spmd
from concourse.masks import make_identity

B = 2048
F = 40
D = 64
NCORES = 8
BS = B // NCORES                   # 256 rows per core
PAIRS = F * (F - 1) // 2           # 780
OUT_W = PAIRS * D                  # 49920
FD = F * D                         # 2560
DT = mybir.dt.float32
HT = mybir.dt.float16

BLOCK_LEN = [F - 1 - i for i in range(F - 1)]
BLOCK_OFF = np.concatenate([[0], np.cumsum(BLOCK_LEN)[:-1]]).tolist()

# x chunk column ranges per tile, loaded in listed (reverse-field) order
CH_T0 = [(2304, 2560), (2048, 2304), (1536, 2048), (1024, 1536),
         (512, 1024), (0, 512)]
CH_T1 = [(1280, 2560), (0, 1280)]


def _chunk_of_field(f: int) -> int:
    for c, (c0, c1) in enumerate(CH_T0):
        if c0 <= f * D < c1:
            return c
    raise AssertionError(f)


def _make_groups():
    """Pack descending leads into contiguous output groups; small groups
    first so the output DMA stream starts early."""
    caps = [512, 1536, 2560, 3584]
    groups, cur, fe = [], [], 0
    for i in range(F - 2, -1, -1):
        f = (F - 1 - i) * D
        cap = caps[len(groups)] if len(groups) < len(caps) else 4608
        if cur and fe + f > cap:
            groups.append(cur)
            cur, fe = [], 0
        cur.append(i)
        fe += f
    if cur:
        groups.append(cur)
    return groups


GROUPS = _make_groups()

# Late groups (big single leads; ACT prep is finished by then) compute in
# fp16-out (DVE 2x mode) + an ACT fp16->f32 upconvert into the staging
# tile. This hedges the intermittent ~20% DVE-slow device mode: it keeps
# DVE production above the DMA drain rate even degraded, while ACT
# (~60us busy) absorbs the ~26us of upconverts with room to spare.
H16_GROUPS = set(range(11, len(GROUPS)))


_CACHE = {}


def _build(bs: int):
    assert bs % 128 == 0
    ntiles = bs // 128
    nc = bacc.Bacc("TRN2", target_bir_lowering=False, debug=False)

    x_dram = nc.dram_tensor("x", [bs, F, D], DT, kind="ExternalInput").ap()
    w_dram = nc.dram_tensor("w", [D, D], DT, kind="ExternalInput").ap()
    out_dram = nc.dram_tensor("out", [bs, OUT_W], DT, kind="ExternalOutput").ap()

    x_flat = x_dram.rearrange("b f d -> b (f d)")

    with tile.TileContext(nc) as tc, ExitStack() as ctx:
        const_pool = ctx.enter_context(tc.tile_pool(name="const", bufs=1))
        x_pool = ctx.enter_context(tc.tile_pool(name="x", bufs=2))
        x16_pool = ctx.enter_context(tc.tile_pool(name="x16", bufs=2))
        xw16_pool = ctx.enter_context(tc.tile_pool(name="xw16", bufs=2))
        tr_pool = ctx.enter_context(tc.tile_pool(name="tr", bufs=3))
        st16_pool = ctx.enter_context(tc.tile_pool(name="st16", bufs=3))
        st32_pool = ctx.enter_context(tc.tile_pool(name="st32", bufs=6))
        psum_tr = ctx.enter_context(tc.tile_pool(name="psum_tr", bufs=2, space="PSUM"))
        psum_mm = ctx.enter_context(tc.tile_pool(name="psum_mm", bufs=4, space="PSUM"))

        ident = const_pool.tile([128, 128], DT)
        make_identity(nc, ident[:])
        # W on both partition halves so the two per-pair matmuls read lhsT
        # and rhs from the same base partition
        w_sb = const_pool.tile([128, D], DT)
        nc.scalar.dma_start(w_sb[0:D, :], w_dram)
        nc.scalar.dma_start(w_sb[D:128, :], w_dram)

        x_tiles = [x_pool.tile([128, FD], DT, name="x_t") for _ in range(ntiles)]
        x16_tiles = [x16_pool.tile([128, FD], HT, name="x16_t") for _ in range(ntiles)]

        # tile0 x: all chunks on the sync ring, ahead of the output writes
        # in that queue. tile1 chunks drip in from the scalar stream below
        # (each dma_start costs the issuing engine ~1.3us, so they must not
        # crowd the ACT stream during the early prep-critical phase).
        # (No gpsimd SWDGE DMAs: they skew DMA engine 79 ~25% slow, and
        # every group-completion semaphore then waits on the straggler.)
        for c0, c1 in CH_T0:
            nc.sync.dma_start(x_tiles[0][:, c0:c1], x_flat[0:128, c0:c1])

        for t in range(ntiles):
            b0 = t * 128
            x_t, x16_t = x_tiles[t], x16_tiles[t]
            xw16 = xw16_pool.tile([128, FD], HT)
            done_fp = set()
            done_ch = set()

            def prep(leads):
                """Emit x16 converts + transpose/matmul/xw16 for a group."""
                for i in leads:
                    ch = _chunk_of_field(i)
                    for c in range(ch + 1):
                        if c not in done_ch:
                            done_ch.add(c)
                            c0, c1 = CH_T0[c]
                            nc.scalar.copy(x16_t[:, c0:c1], x_t[:, c0:c1])
                    fp = i // 2
                    if fp in done_fp:
                        continue
                    done_fp.add(fp)
                    tr_ps = psum_tr.tile([128, 128], DT)
                    nc.tensor.transpose(
                        tr_ps[:], x_t[:, fp * 128 : (fp + 1) * 128], ident[:]
                    )
                    tr_sb = tr_pool.tile([128, 128], DT)
                    nc.scalar.copy(tr_sb[:], tr_ps[:])
                    for h in range(2):
                        fi = 2 * fp + h
                        if fi > F - 2:
                            continue  # xw of field 39 never leads a pair
                        mm = psum_mm.tile([128, D], DT, tag="mm")
                        nc.tensor.matmul(
                            mm[:],
                            tr_sb[h * D : (h + 1) * D, :],
                            w_sb[h * D : (h + 1) * D, :],
                            start=True,
                            stop=True,
                        )
                        nc.scalar.copy(xw16[:, fi * D : (fi + 1) * D], mm[:])

            prep(GROUPS[0])
            if len(GROUPS) > 1:
                prep(GROUPS[1])

            pending = list(CH_T1) if (t == 0 and ntiles > 1) else []
            for g, leads in enumerate(GROUPS):
                if g + 2 < len(GROUPS):
                    prep(GROUPS[g + 2])
                if pending:
                    c0, c1 = pending.pop(0)
                    nc.scalar.dma_start(
                        x_tiles[1][:, c0:c1], x_flat[128:256, c0:c1]
                    )
                lo, hi = leads[-1], leads[0]
                g_off = BLOCK_OFF[lo] * D
                g_fe = (BLOCK_OFF[hi] + BLOCK_LEN[hi]) * D - g_off
                st32 = st32_pool.tile([128, g_fe], DT, name="st32")
                st16 = None
                if g in H16_GROUPS:
                    st16 = st16_pool.tile([128, g_fe], HT, name="st16")
                for i in leads:
                    jn = F - 1 - i
                    s0 = BLOCK_OFF[i] * D - g_off
                    dst = (st16 if st16 is not None else st32)[
                        :, s0 : s0 + jn * D
                    ]
                    in0 = (
                        xw16[:, i * D : (i + 1) * D]
                        .unsqueeze(1)
                        .broadcast_to([128, jn, D])
                    )
                    in1 = x16_t[:, (i + 1) * D : FD].rearrange(
                        "p (j d) -> p j d", d=D
                    )
                    nc.vector.tensor_mul(
                        dst.rearrange("p (j d) -> p j d", d=D), in0, in1
                    )
                if st16 is not None:
                    nc.scalar.copy(st32[:], st16[:])
                nc.sync.dma_start(
                    out_dram[b0 : b0 + 128, g_off : g_off + g_fe], st32[:]
                )

    nc.compile()
    return nc


def _get_nc(bs: int):
    if bs not in _CACHE:
        _CACHE[bs] = _build(bs)
    return _CACHE[bs]


def _run(inputs: np.ndarray, w: np.ndarray, trace: bool = False):
    inputs = np.ascontiguousarray(inputs, dtype=np.float32)
    w = np.ascontiguousarray(w, dtype=np.float32)
    assert inputs.shape == (B, F, D) and w.shape == (D, D)
    nc = _get_nc(BS)
    in_maps = [
        {"x": inputs[c * BS : (c + 1) * BS], "w": w} for c in range(NCORES)
    ]
    res = run_bass_kernel_spmd(nc, in_maps, list(range(NCORES)), trace=trace)
    out = np.concatenate([res.results[c]["out"] for c in range(NCORES)], axis=0)
    return out, res


def kernel(inputs: np.ndarray, w: np.ndarray) -> np.ndarray:
    out, _ = _run(inputs, w)
    return out


# revision 26
# speedup vs baseline: 1.1364x; 1.1364x over previous
"""BiLinearInteractionLayer (bilinear_type='all') Trainium2 Bass kernel.

Contract: kernel(inputs=[2048,40,64] f32, w=[64,64] f32) -> [2048, 49920] f32,
matching

    xw  = einsum('bfd,de->bfe', inputs, w)
    p   = xw[:, I, :] * inputs[:, J, :]   # (I, J) = triu_indices(40, k=1)
    out = p.reshape(B, -1)

Data-parallel over 8 NeuronCores: batch 2048 -> 8 x 256, W replicated.

Roofline: the kernel is HBM-DMA bound -- 51 MB of output writes per core
against a ~420 GB/s per-core DMA-engine ceiling (~125 us of pure queue
time; measured engines run 97-98% busy in steady state). The fight is to
keep every other engine comfortably below that:

  * f32 broadcast multiplies on DVE run at ~1.25 ns/elem/partition
    (no fast mode for 4-byte dtypes) = ~140 us -- they would pace the
    kernel. GpSimd shares DVE's SBUF ports, so offloading muls there is
    a wash (measured: both engines throttle to the same combined rate).
    Instead x and xw are converted once to fp16 (cheap, on ACT) and the
    muls run fp16-in/f32-out: half the read traffic, ~1.05-1.16 ns/elem,
    ~116 us -- production ~440 GB/s, just above the drain rate, with the
    f32 product written straight into the DMA staging tile (no second
    conversion pass). fp16 inputs keep relative error ~6e-4, far inside
    the 2e-2 gate.
  * Consecutive lead-fields are packed into contiguous output GROUPS
    (small groups first, capped ~4.6K f32/partition): one staging tile
    and one wide DMA per group (lines up to 18 KB), 16 DMAs per tile.
  * x streams in reverse-field chunks on the sync ring ahead of the
    write stream (the queue eats the reads while the mul pipeline
    ramps); transposes/matmuls/fp16 copies are emitted two groups ahead
    of the muls; tile1's chunks drip from the ACT stream mid-tile0.
  * No gpsimd SWDGE DMAs, and 7 staging buffers: DMA engine E79
    intermittently runs ~20% slow, and every group-completion semaphore
    (+16, one per engine) otherwise gates DVE on the straggler.
"""

import numpy as np
from contextlib import ExitStack

import concourse.bass as bass  # noqa: F401  (registers engines)
import concourse.bacc as bacc
import concourse.tile as tile
import concourse.mybir as mybir
from concourse.bass_utils import run_bass_kernel_spmd
from concourse.masks import make_identity

B = 2048
F = 40
D = 64
NCORES = 8
BS = B // NCORES                   # 256 rows per core
PAIRS = F * (F - 1) // 2           # 780
OUT_W = PAIRS * D                  # 49920
FD = F * D                         # 2560
DT = mybir.dt.float32
HT = mybir.dt.float16

BLOCK_LEN = [F - 1 - i for i in range(F - 1)]
BLOCK_OFF = np.concatenate([[0], np.cumsum(BLOCK_LEN)[:-1]]).tolist()

# x chunk column ranges per tile, loaded in listed (reverse-field) order
CH_T0 = [(2304, 2560), (2048, 2304), (1536, 2048), (1024, 1536),
         (512, 1024), (0, 512)]
CH_T1 = [(1280, 2560), (0, 1280)]


def _chunk_of_field(f: int) -> int:
    for c, (c0, c1) in enumerate(CH_T0):
        if c0 <= f * D < c1:
            return c
    raise AssertionError(f)


def _make_groups():
    """Pack descending leads into contiguous output groups; small groups
    first so the output DMA stream starts early."""
    caps = [512, 1536, 2560, 3584]
    groups, cur, fe = [], [], 0
    for i in range(F - 2, -1, -1):
        f = (F - 1 - i) * D
        cap = caps[len(groups)] if len(groups) < len(caps) else 4608
        if cur and fe + f > cap:
            groups.append(cur)
            cur, fe = [], 0
        cur.append(i)
        fe += f
    if cur:
        groups.append(cur)
    return groups


GROUPS = _make_groups()


_CACHE = {}


def _build(bs: int):
    assert bs % 128 == 0
    ntiles = bs // 128
    nc = bacc.Bacc("TRN2", target_bir_lowering=False, debug=False)

    x_dram = nc.dram_tensor("x", [bs, F, D], DT, kind="ExternalInput").ap()
    w_dram = nc.dram_tensor("w", [D, D], DT, kind="ExternalInput").ap()
    out_dram = nc.dram_tensor("out", [bs, OUT_W], DT, kind="ExternalOutput").ap()

    x_flat = x_dram.rearrange("b f d -> b (f d)")

    with tile.TileContext(nc) as tc, ExitStack() as ctx:
        const_pool = ctx.enter_context(tc.tile_pool(name="const", bufs=1))
        x_pool = ctx.enter_context(tc.tile_pool(name="x", bufs=2))
        x16_pool = ctx.enter_context(tc.tile_pool(name="x16", bufs=2))
        xw16_pool = ctx.enter_context(tc.tile_pool(name="xw16", bufs=2))
        tr_pool = ctx.enter_context(tc.tile_pool(name="tr", bufs=3))
        st32_pool = ctx.enter_context(tc.tile_pool(name="st32", bufs=7))
        psum_tr = ctx.enter_context(tc.tile_pool(name="psum_tr", bufs=2, space="PSUM"))
        psum_mm = ctx.enter_context(tc.tile_pool(name="psum_mm", bufs=4, space="PSUM"))

        ident = const_pool.tile([128, 128], DT)
        make_identity(nc, ident[:])
        # W on both partition halves so the two per-pair matmuls read lhsT
        # and rhs from the same base partition
        w_sb = const_pool.tile([128, D], DT)
        nc.scalar.dma_start(w_sb[0:D, :], w_dram)
        nc.scalar.dma_start(w_sb[D:128, :], w_dram)

        x_tiles = [x_pool.tile([128, FD], DT, name="x_t") for _ in range(ntiles)]
        x16_tiles = [x16_pool.tile([128, FD], HT, name="x16_t") for _ in range(ntiles)]

        # tile0 x: all chunks on the sync ring, ahead of the output writes
        # in that queue. tile1 chunks drip in from the scalar stream below
        # (each dma_start costs the issuing engine ~1.3us, so they must not
        # crowd the ACT stream during the early prep-critical phase).
        # (No gpsimd SWDGE DMAs: they skew DMA engine 79 ~25% slow, and
        # every group-completion semaphore then waits on the straggler.)
        for c0, c1 in CH_T0:
            nc.sync.dma_start(x_tiles[0][:, c0:c1], x_flat[0:128, c0:c1])

        for t in range(ntiles):
            b0 = t * 128
            x_t, x16_t = x_tiles[t], x16_tiles[t]
            xw16 = xw16_pool.tile([128, FD], HT)
            done_fp = set()
            done_ch = set()

            def prep(leads):
                """Emit x16 converts + transpose/matmul/xw16 for a group."""
                for i in leads:
                    ch = _chunk_of_field(i)
                    for c in range(ch + 1):
                        if c not in done_ch:
                            done_ch.add(c)
                            c0, c1 = CH_T0[c]
                            nc.scalar.copy(x16_t[:, c0:c1], x_t[:, c0:c1])
                    fp = i // 2
                    if fp in done_fp:
                        continue
                    done_fp.add(fp)
                    tr_ps = psum_tr.tile([128, 128], DT)
                    nc.tensor.transpose(
                        tr_ps[:], x_t[:, fp * 128 : (fp + 1) * 128], ident[:]
                    )
                    tr_sb = tr_pool.tile([128, 128], DT)
                    nc.scalar.copy(tr_sb[:], tr_ps[:])
                    for h in range(2):
                        fi = 2 * fp + h
                        if fi > F - 2:
                            continue  # xw of field 39 never leads a pair
                        mm = psum_mm.tile([128, D], DT, tag="mm")
                        nc.tensor.matmul(
                            mm[:],
                            tr_sb[h * D : (h + 1) * D, :],
                            w_sb[h * D : (h + 1) * D, :],
                            start=True,
                            stop=True,
                        )
                        nc.scalar.copy(xw16[:, fi * D : (fi + 1) * D], mm[:])

            prep(GROUPS[0])
            if len(GROUPS) > 1:
                prep(GROUPS[1])

            pending = list(CH_T1) if (t == 0 and ntiles > 1) else []
            for g, leads in enumerate(GROUPS):
                if g + 2 < len(GROUPS):
                    prep(GROUPS[g + 2])
                if pending:
                    c0, c1 = pending.pop(0)
                    nc.scalar.dma_start(
                        x_tiles[1][:, c0:c1], x_flat[128:256, c0:c1]
                    )
                lo, hi = leads[-1], leads[0]
                g_off = BLOCK_OFF[lo] * D
                g_fe = (BLOCK_OFF[hi] + BLOCK_LEN[hi]) * D - g_off
                st32 = st32_pool.tile([128, g_fe], DT, name="st32")
                for i in leads:
                    jn = F - 1 - i
                    s0 = BLOCK_OFF[i] * D - g_off
                    dst = st32[:, s0 : s0 + jn * D]
                    in0 = (
                        xw16[:, i * D : (i + 1) * D]
                        .unsqueeze(1)
                        .broadcast_to([128, jn, D])
                    )
                    in1 = x16_t[:, (i + 1) * D : FD].rearrange(
                        "p (j d) -> p j d", d=D
                    )
                    nc.vector.tensor_mul(
                        dst.rearrange("p (j d) -> p j d", d=D), in0, in1
                    )
                nc.sync.dma_start(
                    out_dram[b0 : b0 + 128, g_off : g_off + g_fe], st32[:]
                )

    nc.compile()
    return nc


def _get_nc(bs: int):
    if bs not in _CACHE:
        _CACHE[bs] = _build(bs)
    return _CACHE[bs]


def _run(inputs: np.ndarray, w: np.ndarray, trace: bool = False):
    inputs = np.ascontiguousarray(inputs, dtype=np.float32)
    w = np.ascontiguousarray(w, dtype=np.float32)
    assert inputs.shape == (B, F, D) and w.shape == (D, D)
    nc = _get_nc(BS)
    in_maps = [
        {"x": inputs[c * BS : (c + 1) * BS], "w": w} for c in range(NCORES)
    ]
    res = run_bass_kernel_spmd(nc, in_maps, list(range(NCORES)), trace=trace)
    out = np.concatenate([res.results[c]["out"] for c in range(NCORES)], axis=0)
    return out, res


def kernel(inputs: np.ndarray, w: np.ndarray) -> np.ndarray:
    out, _ = _run(inputs, w)
    return out
